# revision 3
# baseline (speedup 1.0000x reference)
"""AttLIF Trainium2 kernel (8-core data-parallel SPMD).

Reference computation (per batch shard):
  x = data @ W.T + b                       # Linear [B,T,I]->[B,T,H]
  s = mean_h(x); a = sigmoid(relu(s@w1.T+b1)@w2.T+b2)   # TA gate [B,T]
  x = x * a[:, :, None]
  LIF over T: u = a*u + x_t; sp = (u>=VTH); u *= (1-sp) # hard reset

Strategy:
  - Shard B=128 over 8 cores (16 each); W replicated.
  - Matmul in bf16x3 split (hi*hi + hi*lo + lo*hi accumulated in fp32 PSUM)
    -> near-fp32 accuracy (spikes are threshold crossings; pure bf16 flips
    ~9k spikes, bf16x3 flips ~15 of 16.7M).
  - s computed on-device as dh.T @ mean_h(W) (+mean(b)); TA MLP on 16
    partitions; gate fused into the PSUM drain.
  - LIF: 3 DVE ops per timestep on [128part, 16hc x 8b]; spike output
    overwrites x in place; spikes written to DRAM in device layout and
    transposed back on host.
All host-side work is layout/weight preprocessing only (transposes, bf16
splits, column means of W); every data-dependent FLOP runs on device.
"""

import functools
import numpy as np

ALPHA = 0.3
VTH = 0.3
B, T, I, H = 128, 64, 2048, 2048
NCORES = 8
BL = B // NCORES          # local batch = 16
TOK = BL * T              # 1024 tokens per core
IC = I // 128             # 16 contraction chunks
HC = H // 128             # 16 hidden chunks
NTOKC = 2                 # token chunks of 512 (8 local batches each)
TOKC = TOK // NTOKC       # 512
BC = BL // NTOKC          # 8 batches per token chunk


def _dt():
    import ml_dtypes
    return ml_dtypes.bfloat16


@functools.cache
def _build():
    import sys
    if "/opt/trn_rl_repo" not in sys.path:
        sys.path.insert(0, "/opt/trn_rl_repo")
    from contextlib import ExitStack
    from concourse import bacc, mybir, tile

    f32 = mybir.dt.float32
    bf16 = mybir.dt.bfloat16
    Alu = mybir.AluOpType
    Act = mybir.ActivationFunctionType

    nc = bacc.Bacc("TRN2", target_bir_lowering=False, debug=False)

    dh_d = nc.dram_tensor("dh", [I, TOK], bf16, kind="ExternalInput")
    dl_d = nc.dram_tensor("dl", [I, TOK], bf16, kind="ExternalInput")
    wh_d = nc.dram_tensor("wh", [I, H], bf16, kind="ExternalInput")
    wl_d = nc.dram_tensor("wl", [I, H], bf16, kind="ExternalInput")
    bias_d = nc.dram_tensor("bias", [128, HC], f32, kind="ExternalInput")
    wbar_d = nc.dram_tensor("wbar", [128, IC], bf16, kind="ExternalInput")
    bbar_d = nc.dram_tensor("bbar", [1, 1], f32, kind="ExternalInput")
    w1r_d = nc.dram_tensor("w1r", [BL, 4, T], f32, kind="ExternalInput")
    b1r_d = nc.dram_tensor("b1r", [BL, 4], f32, kind="ExternalInput")
    w2r_d = nc.dram_tensor("w2r", [BL, T, 4], f32, kind="ExternalInput")
    b2r_d = nc.dram_tensor("b2r", [BL, T], f32, kind="ExternalInput")
    spk_d = nc.dram_tensor("spk", [128, HC, BL, T], f32, kind="ExternalOutput")

    s_dram = nc.dram_tensor("s_scratch", [TOK], f32)
    a_dram = nc.dram_tensor("a_scratch", [BL, T], f32)

    with ExitStack() as ctx:
        tc = ctx.enter_context(tile.TileContext(nc))
        const = ctx.enter_context(tc.tile_pool(name="const", bufs=1))
        wpool = ctx.enter_context(tc.tile_pool(name="wpool", bufs=2))
        xpool = ctx.enter_context(tc.tile_pool(name="xpool", bufs=2))
        upool = ctx.enter_context(tc.tile_pool(name="upool", bufs=2))
        psum = ctx.enter_context(tc.tile_pool(name="psum", bufs=4, space="PSUM"))
        psum_s = ctx.enter_context(tc.tile_pool(name="psum_s", bufs=2, space="PSUM"))

        # ---- persistent loads ----
        dh_sb = const.tile([128, IC, TOK], bf16, tag="dh")
        dl_sb = const.tile([128, IC, TOK], bf16, tag="dl")
        nc.sync.dma_start(out=dh_sb, in_=dh_d.ap().rearrange("(ic p) tok -> p ic tok", p=128))
        nc.sync.dma_start(out=dl_sb, in_=dl_d.ap().rearrange("(ic p) tok -> p ic tok", p=128))
        bias_sb = const.tile([128, HC], f32, tag="bias")
        nc.sync.dma_start(out=bias_sb, in_=bias_d.ap())
        wbar_sb = const.tile([128, IC], bf16, tag="wbar")
        nc.sync.dma_start(out=wbar_sb, in_=wbar_d.ap())
        bbar_sb = const.tile([1, 1], f32, tag="bbar")
        nc.sync.dma_start(out=bbar_sb, in_=bbar_d.ap())
        w1r_sb = const.tile([BL, 4, T], f32, tag="w1r")
        nc.sync.dma_start(out=w1r_sb, in_=w1r_d.ap())
        b1r_sb = const.tile([BL, 4], f32, tag="b1r")
        nc.sync.dma_start(out=b1r_sb, in_=b1r_d.ap())
        w2r_sb = const.tile([BL, T, 4], f32, tag="w2r")
        nc.sync.dma_start(out=w2r_sb, in_=w2r_d.ap())
        b2r_sb = const.tile([BL, T], f32, tag="b2r")
        nc.sync.dma_start(out=b2r_sb, in_=b2r_d.ap())

        # ---- squeeze: s[tok] = dh.T @ wbar + bbar  (wbar = mean_h W) ----
        s_sb = const.tile([1, TOK], f32, tag="s")
        for tc_i in range(NTOKC):
            ps = psum_s.tile([1, TOKC], f32, tag="ps_s")
            for ic in range(IC):
                nc.tensor.matmul(
                    ps,
                    lhsT=wbar_sb[:, ic : ic + 1],
                    rhs=dh_sb[:, ic, tc_i * TOKC : (tc_i + 1) * TOKC],
                    start=(ic == 0),
                    stop=(ic == IC - 1),
                )
            nc.vector.tensor_scalar(
                out=s_sb[:, tc_i * TOKC : (tc_i + 1) * TOKC],
                in0=ps, scalar1=bbar_sb, scalar2=None, op0=Alu.add,
            )
        # bounce through DRAM to re-partition [1,1024] -> [16,64]
        nc.sync.dma_start(out=s_dram.ap(), in_=s_sb)
        sT_sb = const.tile([BL, T], f32, tag="sT")
        nc.sync.dma_start(out=sT_sb, in_=s_dram.ap().rearrange("(b t) -> b t", b=BL))

        # ---- TA excite MLP on 16 partitions ----
        h1_sb = const.tile([BL, 4], f32, tag="h1")
        tmp_sb = const.tile([BL, T], f32, tag="ta_tmp")
        for r in range(4):
            nc.vector.tensor_tensor(
                out=tmp_sb, in0=sT_sb, in1=w1r_sb[:, r : r + 1, :], op=Alu.mult
            )
            nc.vector.tensor_reduce(
                out=h1_sb[:, r : r + 1], in_=tmp_sb,
                axis=mybir.AxisListType.X, op=Alu.add,
            )
        h1b_sb = const.tile([BL, 4], f32, tag="h1b")
        nc.vector.tensor_tensor(out=h1b_sb, in0=h1_sb, in1=b1r_sb, op=Alu.add)
        h1c_sb = const.tile([BL, 4], f32, tag="h1c")
        nc.scalar.activation(out=h1c_sb, in_=h1b_sb, func=Act.Relu)
        acc = [
            const.tile([BL, T], f32, tag=f"acc{r}", name=f"acc{r}") for r in range(4)
        ]
        nc.vector.scalar_tensor_tensor(
            out=acc[0], in0=w2r_sb[:, :, 0:1], scalar=h1c_sb[:, 0:1],
            in1=b2r_sb, op0=Alu.mult, op1=Alu.add,
        )
        for r in range(1, 4):
            nc.vector.scalar_tensor_tensor(
                out=acc[r], in0=w2r_sb[:, :, r : r + 1], scalar=h1c_sb[:, r : r + 1],
                in1=acc[r - 1], op0=Alu.mult, op1=Alu.add,
            )
        a16_sb = const.tile([BL, T], f32, tag="a16")
        nc.scalar.activation(out=a16_sb, in_=acc[3], func=Act.Sigmoid)

        # ---- replicate gate to all 128 partitions (DRAM bounce + doubling) ----
        a_rep = const.tile([128, BL, T], f32, tag="a_rep")
        nc.sync.dma_start(out=a_dram.ap(), in_=a16_sb)
        nc.sync.dma_start(out=a_rep[0:1, :, :], in_=a_dram.ap())
        k = 1
        while k < 128:
            nc.sync.dma_start(out=a_rep[k : 2 * k, :, :], in_=a_rep[0:k, :, :])
            k *= 2

        # ---- main matmul (bf16x3) + gate drain + LIF + store ----
        for tc_i in range(NTOKC):
            t0 = tc_i * TOKC
            b0 = tc_i * BC
            x_sb = xpool.tile([128, HC, BC, T], f32, tag="x")
            for hcp in range(HC // 2):
                whl = wpool.tile([128, IC, 256], bf16, tag="wh")
                wll = wpool.tile([128, IC, 256], bf16, tag="wl")
                h0 = hcp * 256
                nc.sync.dma_start(
                    out=whl, in_=wh_d[:, h0 : h0 + 256].rearrange("(ic p) h -> p ic h", p=128)
                )
                nc.sync.dma_start(
                    out=wll, in_=wl_d[:, h0 : h0 + 256].rearrange("(ic p) h -> p ic h", p=128)
                )
                for sub in range(2):
                    hc = hcp * 2 + sub
                    hs = sub * 128
                    ps = psum.tile([128, TOKC], f32, tag="ps_mm")
                    for ic in range(IC):
                        rh = dh_sb[:, ic, t0 : t0 + TOKC]
                        rl = dl_sb[:, ic, t0 : t0 + TOKC]
                        nc.tensor.matmul(ps, lhsT=whl[:, ic, hs : hs + 128], rhs=rh,
                                         start=(ic == 0), stop=False)
                        nc.tensor.matmul(ps, lhsT=whl[:, ic, hs : hs + 128], rhs=rl,
                                         start=False, stop=False)
                        nc.tensor.matmul(ps, lhsT=wll[:, ic, hs : hs + 128], rhs=rh,
                                         start=False, stop=(ic == IC - 1))
                    # x = (psum + bias[h]) * a[b,t]
                    nc.vector.scalar_tensor_tensor(
                        out=x_sb[:, hc, :, :],
                        in0=ps.rearrange("p (b t) -> p b t", b=BC),
                        scalar=bias_sb[:, hc : hc + 1],
                        in1=a_rep[:, b0 : b0 + BC, :],
                        op0=Alu.add, op1=Alu.mult,
                    )
            # LIF over T for this batch chunk; spikes overwrite x in place
            u_a = upool.tile([128, HC, BC], f32, tag="u_a")
            u_b = upool.tile([128, HC, BC], f32, tag="u_b")
            nc.vector.memset(u_a, 0.0)
            for t in range(T):
                x_t = x_sb[:, :, :, t]
                nc.vector.scalar_tensor_tensor(
                    out=u_b, in0=u_a, scalar=ALPHA, in1=x_t,
                    op0=Alu.mult, op1=Alu.add,
                )
                nc.vector.tensor_scalar(
                    out=x_t, in0=u_b, scalar1=VTH, scalar2=None, op0=Alu.is_ge
                )
                nc.vector.scalar_tensor_tensor(
                    out=u_a, in0=u_b, scalar=VTH, in1=u_b,
                    op0=Alu.is_lt, op1=Alu.mult,
                )
            nc.sync.dma_start(out=spk_d[:, :, b0 : b0 + BC, :], in_=x_sb)

    nc.compile()
    return nc


def _host_prep(data, W, b, w1, b1, w2, b2):
    bf16 = _dt()
    data = np.ascontiguousarray(data, dtype=np.float32)
    W = np.ascontiguousarray(W, dtype=np.float32)

    WT = np.ascontiguousarray(W.T)                      # [I, H]
    wh = WT.astype(bf16)
    wl = (WT - wh.astype(np.float32)).astype(bf16)
    bias = np.ascontiguousarray(b.reshape(HC, 128).T, dtype=np.float32)
    wbar = W.mean(axis=0, dtype=np.float64).astype(np.float32)  # [I]
    wbar_t = np.ascontiguousarray(wbar.reshape(IC, 128).T).astype(bf16)
    bbar = np.array([[b.mean(dtype=np.float64)]], dtype=np.float32)
    w1r = np.ascontiguousarray(np.broadcast_to(w1[None], (BL, 4, T)), dtype=np.float32)
    b1r = np.ascontiguousarray(np.broadcast_to(b1[None], (BL, 4)), dtype=np.float32)
    w2r = np.ascontiguousarray(np.broadcast_to(w2[None], (BL, T, 4)), dtype=np.float32)
    b2r = np.ascontiguousarray(np.broadcast_to(b2[None], (BL, T)), dtype=np.float32)

    in_maps = []
    for c in range(NCORES):
        dc = np.ascontiguousarray(
            data[c * BL : (c + 1) * BL].reshape(TOK, I).T
        )                                               # [I, TOK]
        dh = dc.astype(bf16)
        dl = (dc - dh.astype(np.float32)).astype(bf16)
        in_maps.append({
            "dh": dh, "dl": dl, "wh": wh, "wl": wl,
            "bias": bias, "wbar": wbar_t, "bbar": bbar,
            "w1r": w1r, "b1r": b1r, "w2r": w2r, "b2r": b2r,
        })
    return in_maps


def _gather(results):
    outs = []
    for c in range(NCORES):
        spk = results[c]["spk"]                         # [128, HC, BL=16? BC..]
        # device layout [128p, HC, BL, T] -> [BL, T, HC, 128] -> [BL, T, H]
        outs.append(
            np.ascontiguousarray(np.transpose(spk, (2, 3, 1, 0))).reshape(BL, T, H)
        )
    return np.concatenate(outs, axis=0)


def kernel(data, W, b, w1, b1, w2, b2):
    import sys
    if "/opt/trn_rl_repo" not in sys.path:
        sys.path.insert(0, "/opt/trn_rl_repo")
    from concourse.bass_utils import run_bass_kernel_spmd

    nc = _build()
    in_maps = _host_prep(data, W, b, w1, b1, w2, b2)
    res = run_bass_kernel_spmd(nc, in_maps, list(range(NCORES)))
    return _gather(res.results).astype(np.float32)
